# revision 13
# baseline (speedup 1.0000x reference)
"""CausalGCN forward on 8 trn2 NeuronCores (Bass/Tile).

Strategy (graph/data parallel, per sharding hint):
  - Nodes sharded contiguously across 8 cores (6250 each). Edges owned by
    their *destination* core.
  - Per GCN layer: each core computes its shard of m~ = dis * (h @ W),
    AllGathers the full [N,128] message table to every core's DRAM, then
    uses SWDGE dma_gather to fetch the 512B source rows of its ~100k edges.
  - Segmented sum over destinations via PE matmuls: lhsT = 0/1 one-hot
    [128 edge-slots, 128 dests] generated on-device (DVE is_equal against an
    iota tile), rhs = gathered messages, accumulating in PSUM per dest block.
    GCN norm factorizes as norm_e = dis[dst]*dis[src]: dis[src] is folded
    into the table (m~), dis[dst] applied at PSUM eviction.
  - Self loops: identity-matmul of the local m~ tile into the same PSUM.
  - int16 gather indices can't span 50000 rows, so each block's edges are
    split into src<32768 / src>=32768 groups gathered from offset views.
  - BatchNorms: stats on device (ones-vector matmuls + AllReduce), folded
    into the adjacent matmul weights. Attention softmax = sigmoid of logit
    difference. Graph pooling via one-hot matmul on sorted batch ids +
    AllReduce; classifier heads computed replicated on every core.

kernel(**inputs) takes the full unsharded inputs and returns
(xc_logits, xo_logits), both [128, 10] float32.
"""

import contextlib

import numpy as np

import concourse.bass as bass
import concourse.mybir as mybir
import concourse.tile as tile
from concourse import library_config
from concourse.library_overlay import lower_extended_insts
from concourse.bass_utils import run_bass_kernel_spmd

DT = mybir.dt.float32
BF = mybir.dt.bfloat16
P = 128          # SBUF partitions
AF = mybir.ActivationFunctionType
OP = mybir.AluOpType

FULL_CFG = dict(
    N=50000, E=800000, H=128, L=3, C=10, G=128, CORES=8,
    SPLIT=32768, SBB=4, GB=8, EPS=1e-5, NQ=1, SP=False,
)


def _split_excess_waits(nc, max_waits=1):
    """This walrus build rejects instructions carrying more than one sync
    wait command. Hoist excess waits onto same-engine no-ops inserted just
    before the instruction (engine program order preserves semantics)."""
    k = 0
    for f in nc.m.functions:
        for bb in f.blocks:
            il = bb.instructions
            i = 0
            while i < len(il):
                inst = il[i]
                si = inst.sync_info
                waits = list(si.on_wait) if si is not None and si.on_wait else []
                if len(waits) > max_waits:
                    si.on_wait = waits[-max_waits:]
                    for j, w in enumerate(waits[:-max_waits]):
                        nop = mybir.InstNoOp(name=f"I-waitfix-{k}", ins=[], outs=[])
                        k += 1
                        nop.engine = inst.engine
                        nop.sync_info = mybir.SyncInfo(on_wait=[w], on_update=[])
                        nc.register_instruction(nop, overwrite=True)
                        il.insert(i, nop)
                        i += 1
                i += 1
    return k


def _install_tilefix():
    """This walrus build rejects >1 sync wait on an InstDrain; split the
    Tile kernel-tail drain's waits across a chain of drains."""
    from concourse.vector_clock import ScopedClock

    if getattr(tile.TileContext, "_drain_fix_installed", False):
        return

    def _patched(self, tick_clock, wait_clock):
        d = self.nc.sync.drain()
        wait_clock.add_sem_waits(d.ins, ScopedClock({None: tick_clock.global_clock}))
        si = d.ins.sync_info
        waits = list(si.on_wait) if si is not None and si.on_wait else []
        if len(waits) > 1:
            si.on_wait = waits[:1]
            for w in waits[1:]:
                d2 = self.nc.sync.drain()
                if d2.ins.sync_info is None:
                    d2.ins.sync_info = mybir.SyncInfo(on_wait=[w], on_update=[])
                else:
                    d2.ins.sync_info.on_wait = [w]
        self.nc.all_engine_barrier()
        assert self.sems is not None
        popped = self.nc._tile_sem_poison_stack.pop()
        assert popped is self._sem_poison
        self.nc.clear_and_free_semaphores(list(self.sems.allocated().values()))
        self.nc.all_engine_barrier()

    tile.TileContext._drain_and_barrier = _patched
    tile.TileContext._drain_fix_installed = True


# ---------------------------------------------------------------- planning

class Plan:
    """Host-side graph preprocessing: uniform (cross-core) chunk structure
    plus per-core gather indices / one-hot generator data.

    Slot layout (identical on every core; per-core counts padded up to the
    cross-core max, pad slots gather row 0 with one-hot column -1):
      for each superblock of SBB dest blocks:
        [all lo chunks, block-major][all hi chunks, block-major]
    """

    def __init__(self, cfg, edge_index, batch):
        N = cfg["N"]
        CORES, SPLIT, SBB = cfg["CORES"], cfg["SPLIT"], cfg["SBB"]
        NP = N // CORES
        NB = (NP + P - 1) // P
        self.cfg = cfg
        self.NP, self.NB = NP, NB

        row = np.asarray(edge_index[0], dtype=np.int64)
        col = np.asarray(edge_index[1], dtype=np.int64)
        deg = np.bincount(row, minlength=N).astype(np.float32) + 1.0
        self.dis = (1.0 / np.sqrt(deg)).astype(np.float32)

        # group edges by (core, block, half); src-half split keeps gather
        # indices within int16
        core = row // NP
        block = (row % NP) // P
        half = (col >= SPLIT).astype(np.int64)
        gkey = (core * NB + block) * 2 + half
        order = np.argsort(gkey, kind="stable")
        rs, cs = row[order], col[order]
        counts = np.bincount(gkey, minlength=CORES * NB * 2).reshape(CORES, NB, 2)
        starts = np.zeros(CORES * NB * 2 + 1, np.int64)
        np.cumsum(counts.reshape(-1), out=starts[1:])

        # uniform per-block chunk counts: max over cores
        nch = -(-counts // P)  # ceil
        nclo = nch[:, :, 0].max(axis=0)
        nchi = nch[:, :, 1].max(axis=0)
        self.nclo, self.nchi = nclo, nchi
        self.OHMAX = max(int(nclo.max(initial=0)), int(nchi.max(initial=0)), 1)

        # global chunk layout + per-superblock call table
        chunk_block = []
        chunk_of = {}
        superblocks = []  # (blocks, {half: (c0, c1)})
        for sb0 in range(0, NB, SBB):
            blocks = list(range(sb0, min(sb0 + SBB, NB)))
            calls = {}
            for h, nc_arr in ((0, nclo), (1, nchi)):
                c0 = len(chunk_block)
                for b in blocks:
                    chunk_of[(b, h)] = len(chunk_block)
                    chunk_block.extend([b] * int(nc_arr[b]))
                if len(chunk_block) > c0:
                    calls[h] = (c0, len(chunk_block))
            superblocks.append((blocks, calls))
        self.chunk_block = chunk_block
        self.chunk_of = chunk_of
        self.superblocks = superblocks
        self.NCHUNK = len(chunk_block)
        self.NSLOT = self.NCHUNK * P
        self.max_lo_chunks = max(
            (c[0][1] - c[0][0] for _, c in superblocks if 0 in c), default=1)
        self.max_hi_chunks = max(
            (c[1][1] - c[1][0] for _, c in superblocks if 1 in c), default=1)

        # per-core slot data
        idx_flat = np.zeros((CORES, self.NSLOT), np.int16)
        db_flat = np.full((CORES, self.NSLOT), -1.0, np.float32)
        for c in range(CORES):
            for b in range(NB):
                for h in range(2):
                    k = (c * NB + b) * 2 + h
                    n = int(counts[c, b, h])
                    if n == 0:
                        continue
                    s0 = chunk_of[(b, h)] * P
                    e0 = starts[k]
                    src = cs[e0:e0 + n]
                    dst = rs[e0:e0 + n]
                    if h:
                        src = src - SPLIT
                    idx_flat[c, s0:s0 + n] = src.astype(np.int16)
                    db_flat[c, s0:s0 + n] = ((dst % NP) % P).astype(np.float32)

        # wrap indices into the 16-partition layout replicated 8x down
        S16 = self.NSLOT // 16
        w16 = idx_flat.reshape(CORES, S16, 16).transpose(0, 2, 1)  # [C,16,S16]
        self.idx_wrapped = np.ascontiguousarray(
            np.tile(w16, (1, 8, 1)).astype(np.int16))  # [C,128,S16]
        self.S16 = S16
        # db tile [C, 128, NCHUNK]: db_tile[c, p, ch] = db of slot ch*128+p
        self.db_tile = np.ascontiguousarray(
            db_flat.reshape(CORES, self.NCHUNK, P).transpose(0, 2, 1))

        # per-core per-partition columns (tile-major: [p, b])
        NBP = NB * P
        dis_col = np.ones((CORES, NBP), np.float32)
        bg_col = np.full((CORES, NBP), -1.0, np.float32)
        b_np = np.asarray(batch, dtype=np.int64)
        for c in range(CORES):
            dis_col[c, :NP] = self.dis[c * NP:(c + 1) * NP]
            bg_col[c, :NP] = b_np[c * NP:(c + 1) * NP].astype(np.float32)
        self.dis_col = np.ascontiguousarray(
            dis_col.reshape(CORES, NB, P).transpose(0, 2, 1))
        self.bgid_col = np.ascontiguousarray(
            bg_col.reshape(CORES, NB, P).transpose(0, 2, 1))

        G = cfg["G"]
        cnt = np.bincount(b_np, minlength=G).astype(np.float32)
        inv_cnt = (1.0 / np.maximum(cnt, 1.0)).astype(np.float32)
        self.inv_cnt_col = np.zeros((P, 1), np.float32)
        self.inv_cnt_col[:G, 0] = inv_cnt


# ---------------------------------------------------------------- builder

def build_program(plan, cfg):
    _install_tilefix()
    N, Hd, L, C, G = cfg["N"], cfg["H"], cfg["L"], cfg["C"], cfg["G"]
    CORES, SPLIT, SBB, GB = cfg["CORES"], cfg["SPLIT"], cfg["SBB"], cfg["GB"]
    EPS, NQ = cfg["EPS"], cfg["NQ"]
    NP, NB = plan.NP, plan.NB
    NBP = NB * P

    nc = bass.Bass("TRN2", target_bir_lowering=False, debug=False,
                   num_devices=CORES)

    # ---- I/O ----
    x_in = nc.dram_tensor("x", [NP, Hd], DT, kind="ExternalInput")
    idx_in = nc.dram_tensor("idx", [P, plan.S16], mybir.dt.int16, kind="ExternalInput")
    db_in = nc.dram_tensor("db", [P, plan.NCHUNK], BF, kind="ExternalInput")
    iota_in = nc.dram_tensor("iota", [P, plan.OHMAX * P], BF,
                             kind="ExternalInput")
    ident_in = nc.dram_tensor("ident", [P, P], DT, kind="ExternalInput")
    dis_in = nc.dram_tensor("dis_col", [P, NB], DT, kind="ExternalInput")
    bgid_in = nc.dram_tensor("bgid_col", [P, NB], DT, kind="ExternalInput")
    bias_in = nc.dram_tensor("bias_rep", [L + 1, P, Hd], DT, kind="ExternalInput")
    wfeat_in = nc.dram_tensor("w_feat", [Hd, Hd], DT, kind="ExternalInput")
    wconv_in = nc.dram_tensor("w_convs", [max(L, 1), Hd, Hd], DT,
                              kind="ExternalInput")
    wna_in = nc.dram_tensor("w_na", [Hd, 2], DT, kind="ExternalInput")
    wc_in = nc.dram_tensor("w_c", [Hd, C], DT, kind="ExternalInput")
    wo_in = nc.dram_tensor("w_o", [Hd, C], DT, kind="ExternalInput")
    # columns: [bn_feat_g | bn_feat_b | bn_c_g | bn_c_b | bn_o_g | bn_o_b]
    rows_in = nc.dram_tensor("bn_vecs", [Hd, 6], DT, kind="ExternalInput")
    bc_in = nc.dram_tensor("b_c", [1, C], DT, kind="ExternalInput")
    bo_in = nc.dram_tensor("b_o", [1, C], DT, kind="ExternalInput")
    bnad_in = nc.dram_tensor("bnad_col", [P, 1], DT, kind="ExternalInput")
    invcnt_in = nc.dram_tensor("inv_cnt_col", [P, 1], DT, kind="ExternalInput")

    yc_out = nc.dram_tensor("yc", [G, C], DT, kind="ExternalOutput")
    yo_out = nc.dram_tensor("yo", [G, C], DT, kind="ExternalOutput")
    dbg = cfg.get("DEBUG", False)
    if dbg:
        dbg_h = nc.dram_tensor("dbg_h", [P, NBP], DT, kind="ExternalOutput")
        dbg_m = nc.dram_tensor("dbg_m", [P, NBP], DT, kind="ExternalOutput")
        dbg_xg = nc.dram_tensor("dbg_xg", [P, 2 * Hd], DT, kind="ExternalOutput")
        dbg_st = nc.dram_tensor("dbg_st", [P, 2], DT, kind="ExternalOutput")

    with tile.TileContext(nc) as tc:
        nc.gpsimd.load_library(library_config.mlp)
        with contextlib.ExitStack() as ctx:
            dram = ctx.enter_context(tc.tile_pool(name="dram", bufs=1, space="DRAM"))
            const = ctx.enter_context(tc.tile_pool(name="const", bufs=1))
            glo_p = ctx.enter_context(tc.tile_pool(name="glo", bufs=2))
            ghi_p = ctx.enter_context(tc.tile_pool(name="ghi", bufs=2))
            oh_p = ctx.enter_context(tc.tile_pool(name="oh", bufs=4))
            sc_p = ctx.enter_context(tc.tile_pool(name="scratch", bufs=4))
            ps_work = ctx.enter_context(
                tc.tile_pool(name="ps_work", bufs=3, space="PSUM"))
            ps_agg = ctx.enter_context(
                tc.tile_pool(name="ps_agg", bufs=3, space="PSUM"))
            ps_sm = ctx.enter_context(
                tc.tile_pool(name="ps_sm", bufs=1, space="PSUM"))

            # ---- DRAM internals ----
            ag_in = dram.tile([NP, Hd], BF)
            table = nc.dram_tensor("mtable", [N, Hd], BF, addr_space="Shared")
            ar1_in = dram.tile([P, 2], DT)
            ar1_out = nc.dram_tensor("ar1_out", [P, 2], DT, addr_space="Shared")
            ar2_in = dram.tile([P, 2 * Hd], DT)
            ar2_out = nc.dram_tensor("ar2_out", [P, 2 * Hd], DT, addr_space="Shared")

            # ---- resident SBUF ----
            h_sb = const.tile([P, NBP], DT)
            m_sb = const.tile([P, NBP], BF)
            idx_sb = const.tile([P, plan.S16], mybir.dt.int16)
            db_sb = const.tile([P, plan.NCHUNK], BF)
            iota_sb = const.tile([P, plan.OHMAX * P], BF)
            ident_sb = const.tile([P, P], DT)
            identb_sb = const.tile([P, P], BF)
            dis_sb = const.tile([P, NB], DT)
            bgid_sb = const.tile([P, NB], DT)
            bias_sb = const.tile([P, (L + 1) * Hd], DT)
            wfeat_sb = const.tile([P, Hd], DT)
            wfeat_eff = const.tile([P, Hd], DT)      # BN-folded W_feat
            wconv_sb = const.tile([P, max(L, 1) * Hd], DT)
            wna_sb = const.tile([P, 2], DT)
            wc_sb = const.tile([P, C], DT)
            wo_sb = const.tile([P, C], DT)
            whead_eff = const.tile([P, 2 * C], DT)   # BN-folded W_c | W_o
            bnvec_sb = const.tile([Hd, 6], DT)
            bc_sb = const.tile([1, C], DT)
            bo_sb = const.tile([1, C], DT)
            bnad_sb = const.tile([P, 1], DT)
            invcnt_sb = const.tile([P, 1], DT)
            iota32_sb = const.tile([P, P], DT)       # fp32 iota for head one-hot
            ones_col = const.tile([P, 1], DT)
            ones_row = const.tile([1, P], DT)
            eps_col = const.tile([P, 1], DT)
            vcol = const.tile([P, 8], DT)            # column-vector workspace
            biasp_row = const.tile([1, Hd], DT)      # layer-0 folded BN bias @ W
            stats_sb = const.tile([P, 2], DT)
            stats_red = const.tile([P, 2], DT)
            pooled_sb = const.tile([P, 2 * Hd], DT)
            pooled_red = const.tile([P, 2 * Hd], DT)
            xg_sb = const.tile([P, 2 * Hd], DT)      # pooled, count-normalized
            bias2_row = const.tile([1, 2 * C], DT)   # folded head biases
            out_sb = const.tile([P, 2 * C], DT)

            # ---- loads ----
            nc.vector.memset(h_sb[:], 0.0)
            nc.vector.memset(m_sb[:], 0.0)
            nc.gpsimd.memset(ones_col[:], 1.0)
            nc.gpsimd.memset(ones_row[:], 1.0)
            nc.gpsimd.memset(eps_col[:], EPS)
            nc.sync.dma_start(idx_sb[:], idx_in[:, :])
            nc.sync.dma_start(db_sb[:], db_in[:, :])
            nc.sync.dma_start(iota_sb[:], iota_in[:, :])
            nc.sync.dma_start(ident_sb[:], ident_in[:, :])
            nc.vector.tensor_copy(identb_sb[:], ident_sb[:])
            nc.vector.tensor_copy(iota32_sb[:], iota_sb[:, 0:P])
            nc.sync.dma_start(dis_sb[:], dis_in[:, :])
            nc.sync.dma_start(bgid_sb[:], bgid_in[:, :])
            nc.sync.dma_start(
                bias_sb[:].rearrange("p (l f) -> p l f", f=Hd),
                bias_in.ap().rearrange("l p f -> p l f"))
            nc.sync.dma_start(wfeat_sb[:], wfeat_in[:, :])
            nc.sync.dma_start(
                wconv_sb[:].rearrange("k (l f) -> k l f", f=Hd),
                wconv_in.ap().rearrange("l k f -> k l f"))
            nc.sync.dma_start(wna_sb[:], wna_in[:, :])
            nc.sync.dma_start(wc_sb[:], wc_in[:, :])
            nc.sync.dma_start(wo_sb[:], wo_in[:, :])
            nc.sync.dma_start(bnvec_sb[:], rows_in[:, :])
            nc.sync.dma_start(bc_sb[:], bc_in[:, :])
            nc.sync.dma_start(bo_sb[:], bo_in[:, :])
            nc.sync.dma_start(bnad_sb[:], bnad_in[:, :])
            nc.sync.dma_start(invcnt_sb[:], invcnt_in[:, :])

            # x -> h_sb tiles  (h_sb[p, b*H+f] = x[b*128+p, f])
            nfull = NP // P
            if nfull:
                nc.sync.dma_start(
                    h_sb[:].rearrange("p (b f) -> p b f", f=Hd)[:, :nfull, :],
                    x_in.ap()[0:nfull * P, :].rearrange("(b p) f -> p b f", p=P))
            if NP % P:
                nc.sync.dma_start(h_sb[0:NP % P, nfull * Hd:(nfull + 1) * Hd],
                                  x_in.ap()[nfull * P:NP, :])

            _reg_cache = {}

            def cnt_reg_for(nidx):
                if nidx not in _reg_cache:
                    _reg_cache[nidx] = nc.gpsimd.to_reg(nidx)
                return _reg_cache[nidx]

            def h_tile(b):
                return h_sb[:, b * Hd:(b + 1) * Hd]

            def m_tile(b):
                return m_sb[:, b * Hd:(b + 1) * Hd]

            # ---- input BN stats as feature-columns: h.T @ ones ----
            # two psum tiles: one pending accumulation group per bank
            st_ps = ps_sm.tile([P, 1], DT, tag="sm")
            sq_ps = ps_sm.tile([P, 1], DT, tag="sm2")
            for b in range(NB):
                sq = sc_p.tile([P, Hd], DT, tag="sq")
                nc.vector.tensor_mul(sq[:], h_tile(b), h_tile(b))
                nc.tensor.matmul(st_ps[:], h_tile(b), ones_col[:],
                                 start=(b == 0), stop=(b == NB - 1))
                nc.tensor.matmul(sq_ps[:], sq[:], ones_col[:],
                                 start=(b == 0), stop=(b == NB - 1))
            nc.vector.tensor_copy(stats_sb[:, 0:1], st_ps[:])
            nc.vector.tensor_copy(stats_sb[:, 1:2], sq_ps[:])
            nc.gpsimd.dma_start(ar1_in[:], stats_sb[:])
            nc.gpsimd.collective_compute(
                "AllReduce", OP.add, replica_groups=[list(range(CORES))],
                ins=[ar1_in.opt()], outs=[ar1_out.ap().opt()])
            nc.gpsimd.dma_start(stats_red[:], ar1_out.ap())

            # ---- fold input BN into W_feat (all in column space) ----
            mu_c, var_c, s_col, shift_col, tmp_c = (
                vcol[:, i:i + 1] for i in range(5))
            nc.vector.tensor_scalar_mul(mu_c, stats_red[:, 0:1], 1.0 / N)
            nc.vector.tensor_scalar_mul(var_c, stats_red[:, 1:2], 1.0 / N)
            nc.vector.tensor_mul(tmp_c, mu_c, mu_c)
            nc.vector.tensor_sub(var_c, var_c, tmp_c)          # E[x^2]-mu^2
            nc.scalar.activation(tmp_c, var_c, AF.Sqrt, bias=eps_col[:])
            nc.vector.reciprocal(tmp_c, tmp_c)                 # rsqrt(var+eps)
            nc.vector.tensor_mul(s_col, tmp_c, bnvec_sb[:, 0:1])       # * gamma
            nc.vector.tensor_mul(tmp_c, mu_c, s_col)
            nc.vector.tensor_sub(shift_col, bnvec_sb[:, 1:2], tmp_c)   # beta-mu*s
            nc.vector.tensor_scalar_mul(wfeat_eff[:], wfeat_sb[:], s_col)
            bp_ps = ps_sm.tile([1, Hd], DT, tag="sm")
            nc.tensor.matmul(bp_ps[:], shift_col, wfeat_sb[:])
            nc.vector.tensor_copy(biasp_row[:], bp_ps[:])

            # ---- GCN layers ----
            nlayers = int(cfg.get("NLAYERS", L + 1))
            for layer in range(nlayers):
                wi = (layer - 1) % L if layer > 0 else 0
                w_ap = wfeat_eff[:] if layer == 0 \
                    else wconv_sb[:, wi * Hd:(wi + 1) * Hd]
                # m~ = dis * (h @ W)  [+ BN-shift bias fold on layer 0]
                for b in range(NB):
                    trp = ps_work.tile([P, P], DT, tag="w")
                    nc.tensor.transpose(trp[:], h_tile(b), ident_sb[:])
                    hT = sc_p.tile([P, P], DT, tag="hT")
                    nc.vector.tensor_copy(hT[:], trp[:])
                    mp = ps_work.tile([P, Hd], DT, tag="w")
                    if layer == 0:
                        nc.tensor.matmul(mp[:], hT[:], w_ap, start=True, stop=False)
                        nc.tensor.matmul(mp[:], ones_row[:], biasp_row[:],
                                         start=False, stop=True)
                    else:
                        nc.tensor.matmul(mp[:], hT[:], w_ap)
                    nc.vector.tensor_scalar_mul(m_tile(b), mp[:], dis_sb[:, b:b + 1])

                # publish the message table
                if nfull:
                    nc.sync.dma_start(
                        ag_in[0:nfull * P, :].rearrange("(b p) f -> p b f", p=P),
                        m_sb[:].rearrange("p (b f) -> p b f", f=Hd)[:, :nfull, :])
                if NP % P:
                    nc.sync.dma_start(ag_in[nfull * P:NP, :],
                                      m_sb[0:NP % P, nfull * Hd:(nfull + 1) * Hd])
                if not cfg.get("SKIP_AG", False):
                    nc.gpsimd.collective_compute(
                        "AllGather", OP.bypass,
                        replica_groups=[list(range(CORES))],
                        ins=[ag_in.opt()], outs=[table.ap().opt()])

                # aggregate per superblock
                qn = 0
                for blocks, calls in plan.superblocks:
                    gts = {}
                    for halfsel in (0, 1):
                        if halfsel not in calls:
                            continue
                        c0, c1 = calls[halfsel]
                        nch = c1 - c0
                        pool, tag, cap = (glo_p, "glo", plan.max_lo_chunks) \
                            if halfsel == 0 else (ghi_p, "ghi", plan.max_hi_chunks)
                        gt = pool.tile([P, cap, Hd], BF, tag=tag)
                        src_ap = table.ap()[0:min(SPLIT, N), :] if halfsel == 0 \
                            else table.ap()[SPLIT:N, :]
                        nidx = nch * P
                        if not cfg.get("SKIP_GATHER", False):
                            nc.gpsimd.dma_gather(
                                gt[:, 0:nch, :], src_ap,
                                idx_sb[:, c0 * 8:c1 * 8],
                                nidx, cnt_reg_for(nidx), Hd, queue_num=qn,
                                single_packet=bool(cfg.get("SP", True)))
                        else:
                            nc.vector.memset(gt[:, 0:1, 0:4], 0.0)
                        qn = (qn + 1) % NQ
                        gts[halfsel] = (gt, c0, c1)

                    # block-major: one-hot gen + matmuls in consumption
                    # order (keeps the DVE queue free of slot deadlocks);
                    # one psum group open at a time, one bank per block
                    for b in blocks:
                        agg = ps_agg.tile([P, Hd], DT, tag="agg")
                        first = True
                        for halfsel in (0, 1):
                            if halfsel not in gts:
                                continue
                            gt, c0, c1 = gts[halfsel]
                            nbc = int((plan.nclo if halfsel == 0
                                       else plan.nchi)[b])
                            if nbc == 0:
                                continue
                            b0c = plan.chunk_of[(b, halfsel)]
                            oh = oh_p.tile([P, plan.OHMAX * P], BF, tag="oh")
                            if cfg.get("SKIP_OH", False):
                                nc.vector.memset(oh[:, 0:4], 0.0)
                            elif True:
                                pass
                            if not cfg.get("SKIP_OH", False):
                                nc.vector.tensor_tensor(
                                    oh[:].rearrange(
                                        "p (g f) -> p g f", f=P)[:, 0:nbc, :],
                                    iota_sb[:].rearrange(
                                        "p (g f) -> p g f", f=P)[:, 0:nbc, :],
                                    db_sb[:, b0c:b0c + nbc].broadcast_to(
                                        [P, nbc, P]),
                                    OP.is_equal)
                            for j in range(nbc):
                                if cfg.get("SKIP_MM", False):
                                    continue
                                nc.tensor.matmul(
                                    agg[:],
                                    oh[:, j * P:(j + 1) * P],
                                    gt[:, b0c - c0 + j, :],
                                    start=first, stop=False)
                                first = False
                        nc.tensor.matmul(agg[:], identb_sb[:], m_tile(b),
                                         start=first, stop=True)
                        dst = h_tile(b)
                        nc.vector.scalar_tensor_tensor(
                            dst, agg[:],
                            dis_sb[:, b:b + 1],
                            bias_sb[:, (layer % (L + 1)) * Hd:
                                    (layer % (L + 1) + 1) * Hd],
                            OP.mult, OP.add)
                        nc.scalar.activation(dst, dst, AF.Relu)

            if dbg:
                nc.sync.dma_start(dbg_h[:, :], h_sb[:])
                nc.sync.dma_start(dbg_m[:, :], m_sb[:])
                nc.sync.dma_start(dbg_st[:, :], stats_red[:])

            # ---- head: node attention + graph pooling ----
            att_ps = ps_sm.tile([P, 2 * NB], DT, tag="sm")
            pool_ps_c = ps_agg.tile([P, Hd], DT, tag="agg")
            pool_ps_o = ps_agg.tile([P, Hd], DT, tag="agg")
            for b in range(NB):
                trp = ps_work.tile([P, P], DT, tag="w")
                nc.tensor.transpose(trp[:], h_tile(b), ident_sb[:])
                hT = sc_p.tile([P, P], DT, tag="hT")
                nc.vector.tensor_copy(hT[:], trp[:])
                nc.tensor.matmul(att_ps[:, 2 * b:2 * b + 2], hT[:], wna_sb[:])
                att_t = sc_p.tile([P, 2], DT, tag="attt")
                nc.vector.tensor_copy(att_t[:], att_ps[:, 2 * b:2 * b + 2])
                d_col = sc_p.tile([P, 1], DT, tag="dcol")
                nc.vector.tensor_sub(d_col[:], att_t[:, 0:1], att_t[:, 1:2])
                nc.vector.tensor_scalar_add(d_col[:], d_col[:], bnad_sb[:])
                a0 = sc_p.tile([P, 1], DT, tag="a0")
                nc.scalar.activation(a0[:], d_col[:], AF.Sigmoid)
                a1 = sc_p.tile([P, 1], DT, tag="a1")
                nc.vector.tensor_scalar(a1[:], a0[:], -1.0, 1.0, OP.mult, OP.add)
                xc_t = sc_p.tile([P, Hd], DT, tag="xct")
                nc.vector.tensor_scalar_mul(xc_t[:], h_tile(b), a0[:])
                xo_t = sc_p.tile([P, Hd], DT, tag="xot")
                nc.vector.tensor_scalar_mul(xo_t[:], h_tile(b), a1[:])
                oB = sc_p.tile([P, P], DT, tag="oB")
                nc.vector.tensor_tensor(
                    oB[:], iota32_sb[:],
                    bgid_sb[:, b:b + 1].broadcast_to([P, P]), OP.is_equal)
                nc.tensor.matmul(pool_ps_c[:], oB[:], xc_t[:],
                                 start=(b == 0), stop=(b == NB - 1))
                nc.tensor.matmul(pool_ps_o[:], oB[:], xo_t[:],
                                 start=(b == 0), stop=(b == NB - 1))
            nc.vector.tensor_copy(pooled_sb[:, 0:Hd], pool_ps_c[:])
            nc.vector.tensor_copy(pooled_sb[:, Hd:2 * Hd], pool_ps_o[:])
            nc.gpsimd.dma_start(ar2_in[:], pooled_sb[:])
            nc.gpsimd.collective_compute(
                "AllReduce", OP.add, replica_groups=[list(range(CORES))],
                ins=[ar2_in.opt()], outs=[ar2_out.ap().opt()])
            nc.gpsimd.dma_start(pooled_red[:], ar2_out.ap())

            # count-normalize both heads at once (graph id on partition axis)
            nc.vector.tensor_scalar_mul(xg_sb[:], pooled_red[:], invcnt_sb[:])

            if dbg:
                nc.sync.dma_start(dbg_xg[:, :], xg_sb[:])

            # head BN per head: column-space stats over the G graphs
            for hi_, (w_sb, brow) in enumerate(((wc_sb, bc_sb), (wo_sb, bo_sb))):
                xg_sl = xg_sb[:, hi_ * Hd:(hi_ + 1) * Hd]
                sq2 = sc_p.tile([P, Hd], DT, tag="sq")
                nc.vector.tensor_mul(sq2[:], xg_sl, xg_sl)
                st2_ps = ps_sm.tile([P, 1], DT, tag="sm")
                sq2_ps = ps_sm.tile([P, 1], DT, tag="sm2")
                nc.tensor.matmul(st2_ps[:], xg_sl, ones_col[:])
                nc.tensor.matmul(sq2_ps[:], sq2[:], ones_col[:])
                g_col = bnvec_sb[:, 2 + 2 * hi_:3 + 2 * hi_]
                bcol = bnvec_sb[:, 3 + 2 * hi_:4 + 2 * hi_]
                mu2, var2, s2_col, sh2_col, t2 = (
                    vcol[:, i:i + 1] for i in range(3, 8))
                nc.vector.tensor_scalar_mul(mu2, st2_ps[:], 1.0 / G)
                nc.vector.tensor_scalar_mul(var2, sq2_ps[:], 1.0 / G)
                nc.vector.tensor_mul(t2, mu2, mu2)
                nc.vector.tensor_sub(var2, var2, t2)
                nc.scalar.activation(t2, var2, AF.Sqrt, bias=eps_col[:])
                nc.vector.reciprocal(t2, t2)
                nc.vector.tensor_mul(s2_col, t2, g_col)
                nc.vector.tensor_mul(t2, mu2, s2_col)
                nc.vector.tensor_sub(sh2_col, bcol, t2)
                nc.vector.tensor_scalar_mul(whead_eff[:, hi_ * C:(hi_ + 1) * C],
                                            w_sb[:], s2_col)
                b2_ps = ps_sm.tile([1, C], DT, tag="sm")
                nc.tensor.matmul(b2_ps[:], sh2_col, w_sb[:])
                nc.vector.tensor_add(bias2_row[:, hi_ * C:(hi_ + 1) * C],
                                     b2_ps[:], brow[:])
                # logits = BN(xg) @ W + b  ==  xg @ W_eff + bias2
                trp3 = ps_work.tile([P, P], DT, tag="w")
                nc.tensor.transpose(trp3[:], xg_sb[:, hi_ * Hd:(hi_ + 1) * Hd],
                                    ident_sb[:])
                xgT = sc_p.tile([P, P], DT, tag="hT")
                nc.vector.tensor_copy(xgT[:], trp3[:])
                lg_ps = ps_sm.tile([P, C], DT, tag="sm")
                nc.tensor.matmul(lg_ps[0:G, :], xgT[:, 0:G],
                                 whead_eff[:, hi_ * C:(hi_ + 1) * C],
                                 start=True, stop=False)
                nc.tensor.matmul(lg_ps[0:G, :], ones_row[:, 0:G],
                                 bias2_row[:, hi_ * C:(hi_ + 1) * C],
                                 start=False, stop=True)
                nc.vector.tensor_copy(out_sb[0:G, hi_ * C:(hi_ + 1) * C],
                                      lg_ps[0:G, :])
            nc.sync.dma_start(yc_out[:, :], out_sb[0:G, 0:C])
            nc.sync.dma_start(yo_out[:, :], out_sb[0:G, C:2 * C])

    _split_excess_waits(nc)
    lower_extended_insts(nc)
    return nc


# ---------------------------------------------------------------- host glue

def make_in_maps(plan, cfg, inputs):
    Hd, L, C = cfg["H"], cfg["L"], cfg["C"]
    CORES = cfg["CORES"]
    NP = plan.NP
    import ml_dtypes
    bf16 = ml_dtypes.bfloat16
    x = np.asarray(inputs["x"], np.float32)
    iota = np.ascontiguousarray(
        np.tile(np.arange(P, dtype=np.float32), (P, plan.OHMAX))).astype(bf16)
    ident = np.eye(P, dtype=np.float32)
    bias_rep = np.ascontiguousarray(np.stack(
        [np.broadcast_to(np.asarray(inputs["b_feat"], np.float32), (P, Hd))]
        + [np.broadcast_to(np.asarray(inputs["b_convs"][l], np.float32), (P, Hd))
           for l in range(L)])).astype(np.float32)
    rows = np.ascontiguousarray(np.stack([
        np.asarray(inputs["bn_feat_g"], np.float32),
        np.asarray(inputs["bn_feat_b"], np.float32),
        np.asarray(inputs["bn_c_g"], np.float32),
        np.asarray(inputs["bn_c_b"], np.float32),
        np.asarray(inputs["bn_o_g"], np.float32),
        np.asarray(inputs["bn_o_b"], np.float32),
    ], axis=1))
    b_na = np.asarray(inputs["b_na"], np.float32)
    bnad = np.full((P, 1), float(b_na[0] - b_na[1]), np.float32)
    maps = []
    for c in range(CORES):
        maps.append({
            "x": np.ascontiguousarray(x[c * NP:(c + 1) * NP]),
            "idx": plan.idx_wrapped[c],
            "db": plan.db_tile[c].astype(bf16),
            "iota": iota,
            "ident": ident,
            "dis_col": plan.dis_col[c],
            "bgid_col": plan.bgid_col[c],
            "bias_rep": bias_rep,
            "w_feat": np.ascontiguousarray(np.asarray(inputs["W_feat"], np.float32)),
            "w_convs": np.ascontiguousarray(np.asarray(inputs["W_convs"], np.float32)),
            "w_na": np.ascontiguousarray(np.asarray(inputs["W_na"], np.float32)),
            "w_c": np.ascontiguousarray(np.asarray(inputs["W_c"], np.float32)),
            "w_o": np.ascontiguousarray(np.asarray(inputs["W_o"], np.float32)),
            "bn_vecs": rows,
            "b_c": np.asarray(inputs["b_c"], np.float32).reshape(1, C),
            "b_o": np.asarray(inputs["b_o"], np.float32).reshape(1, C),
            "bnad_col": bnad,
            "inv_cnt_col": plan.inv_cnt_col,
        })
    return maps


def build(inputs, cfg=None):
    cfg = dict(FULL_CFG if cfg is None else cfg)
    plan = Plan(cfg, np.asarray(inputs["edge_index"]), np.asarray(inputs["batch"]))
    nc = build_program(plan, cfg)
    in_maps = make_in_maps(plan, cfg, inputs)
    return nc, in_maps, cfg


class Compiled:
    """Build once, run many times on the 8 NeuronCores via PJRT shard_map.

    Inputs are pinned to the devices once; run() re-executes the compiled
    NEFF without re-shipping them, so wall-clock deltas approximate device
    execution time.
    """

    def __init__(self, inputs, cfg=None):
        import jax
        from jax.sharding import Mesh, PartitionSpec, NamedSharding
        from jax.experimental.shard_map import shard_map
        from concourse import bass2jax

        nc, in_maps, cfg = build(inputs, cfg)
        self.cfg = cfg
        n_cores = cfg["CORES"]
        bass2jax.install_neuronx_cc_hook()

        in_names, out_names, out_avals, zero_outs = [], [], [], []
        part_name = nc.partition_id_tensor.name if nc.partition_id_tensor else None
        for alloc in nc.m.functions[0].allocations:
            if not isinstance(alloc, mybir.MemoryLocationSet):
                continue
            name = alloc.memorylocations[0].name
            if alloc.kind == "ExternalInput":
                if name != part_name:
                    in_names.append(name)
            elif alloc.kind == "ExternalOutput":
                shape = tuple(alloc.tensor_shape)
                dtype = mybir.dt.np(alloc.dtype)
                out_names.append(name)
                out_avals.append(jax.core.ShapedArray(shape, dtype))
                zero_outs.append(np.zeros(shape, dtype))
        n_params = len(in_names)
        n_outs = len(out_names)
        all_names = in_names + out_names
        if part_name is not None:
            all_names.append(part_name)
        donate = tuple(range(n_params, n_params + n_outs))

        def _body(*args):
            operands = list(args)
            if part_name is not None:
                operands.append(bass2jax.partition_id_tensor())
            outs = bass2jax._bass_exec_p.bind(
                *operands,
                out_avals=tuple(out_avals),
                in_names=tuple(all_names),
                out_names=tuple(out_names),
                lowering_input_output_aliases=(),
                sim_require_finite=True,
                sim_require_nnan=True,
                nc=nc,
            )
            return tuple(outs)

        devices = jax.devices()[:n_cores]
        mesh = Mesh(np.asarray(devices), ("core",))
        in_specs = (PartitionSpec("core"),) * (n_params + n_outs)
        out_specs = (PartitionSpec("core"),) * n_outs
        self._fn = jax.jit(
            shard_map(_body, mesh=mesh, in_specs=in_specs,
                      out_specs=out_specs, check_rep=False),
            donate_argnums=donate, keep_unused=True)
        concat_in = [
            np.concatenate([np.asarray(in_maps[c][nm]) for c in range(n_cores)],
                           axis=0)
            for nm in in_names]
        shard = NamedSharding(mesh, PartitionSpec("core"))
        self._dev_in = [jax.device_put(a, shard) for a in concat_in]
        self._zero_outs = zero_outs
        self._out_names = out_names
        self._out_avals = out_avals
        self._n_cores = n_cores
        self._jax = jax
        self._shard = shard

    def run(self):
        jax = self._jax
        zo = [jax.device_put(
            np.zeros((self._n_cores * z.shape[0], *z.shape[1:]), z.dtype),
            self._shard) for z in self._zero_outs]
        outs = jax.block_until_ready(self._fn(*self._dev_in, *zo))
        return {nm: np.asarray(outs[i]).reshape(
                    self._n_cores, *self._out_avals[i].shape)[0]
                for i, nm in enumerate(self._out_names)}

    def time_ns(self, reps=10):
        import time
        # warmup already happened if run() was called; do one more
        self.run()
        best = float("inf")
        for _ in range(reps):
            zo = [self._jax.device_put(
                np.zeros((self._n_cores * z.shape[0], *z.shape[1:]), z.dtype),
                self._shard) for z in self._zero_outs]
            t0 = time.perf_counter_ns()
            self._jax.block_until_ready(self._fn(*self._dev_in, *zo))
            best = min(best, time.perf_counter_ns() - t0)
        return best


def run(inputs, cfg=None):
    nc, in_maps, cfg = build(inputs, cfg)
    res = run_bass_kernel_spmd(nc, in_maps, core_ids=list(range(cfg["CORES"])))
    r0 = res.results[0]
    return (r0["yc"].astype(np.float32), r0["yo"].astype(np.float32))


def kernel(**inputs):
    c = Compiled(inputs)
    r = c.run()
    return (r["yc"].astype(np.float32), r["yo"].astype(np.float32))

